# revision 12
# baseline (speedup 1.0000x reference)
"""Trainium2 Bass kernel for MemoryEfficientMultiHeadAttention (8 NeuronCores).

Sharding: hybrid data/tensor parallel. Core c handles batch b = c//2 and head
group half = c%2 (8 of 16 heads, i.e. 512 of 1024 qkv features). Each core:
  q,k  = (x_b @ w.T + b) in [feat, tok] layout (feat on partitions)
  vT   = (x_b @ wv.T + b) in [tok, feat] layout, with a ones column appended
         per head so the attended matmul also produces the softmax denominator
  per head pair: scoresT = k_h.T @ q_h   (transposed scores, [kt, qt]; the two
                 heads run as row-tiled concurrent matmuls)
            PT = exp(scoresT / 8)        (no max-subtraction: scores are O(1))
            attU[0:64] / denom[64] += [vT_h | 1].T @ PT   (M=65, over kt tiles)
  attS = attU * (1/denom)  broadcast to partitions via gpsimd
  outp = attS.T @ dense_w_slice.T         (partial over this core's 512 feats)
Host: out[b] = outp[2b] + outp[2b+1] + dense_b.

The schedule targets the ACT engine's exp throughput (the roofline for this
decomposition): scores for iteration kt+1 are issued before the attended
matmuls of iteration kt so the PE never head-blocks the exp stream, and the
q/k projections of the next head pair plus the dense output matmuls are
drip-fed into the PE's idle slots (one ~4-matmul burst per odd kt iteration).

All matmuls run in bf16 (1 cycle/row on TRN2 PE; fp32 is 4 cycles/row) with
fp32 PSUM accumulation.
"""

import sys
import time
from contextlib import ExitStack

import numpy as np

try:
    import concourse.bass as bass  # noqa: F401
except ImportError:  # pragma: no cover
    sys.path.insert(0, "/opt/trn_rl_repo")

import ml_dtypes

import concourse.bacc as bacc
import concourse.mybir as mybir
import concourse.tile as tile

P = 128
BF16 = mybir.dt.bfloat16
FP8 = mybir.dt.float8e4
F32 = mybir.dt.float32
NPBF16 = ml_dtypes.bfloat16

# exp output / V dtype. fp8e4m3 would halve ACT SBUF-write and PE rhs-read
# traffic, but measures ~2e-2 rel err (attention outputs are themselves
# softmax averages, so quantization noise is not averaged down) — keep bf16.
PT_DT = BF16

B, S, D = 4, 2048, 1024
HHALF = 512  # features per core (8 heads x 64)


def _build_nc(loop_r=None):
    nc = bacc.Bacc()

    xT = nc.dram_tensor("xT", [D, S], BF16, kind="ExternalInput")
    wqT = nc.dram_tensor("wqT", [D, HHALF], BF16, kind="ExternalInput")
    wkT = nc.dram_tensor("wkT", [D, HHALF], BF16, kind="ExternalInput")
    wvT = nc.dram_tensor("wvT", [D, HHALF], BF16, kind="ExternalInput")
    dwT = nc.dram_tensor("dwT", [HHALF, D], BF16, kind="ExternalInput")
    qb = nc.dram_tensor("qb", [P, 4], F32, kind="ExternalInput")
    kb = nc.dram_tensor("kb", [P, 4], F32, kind="ExternalInput")
    vb = nc.dram_tensor("vb", [P, 8, 64], BF16, kind="ExternalInput")
    outp = nc.dram_tensor("outp", [S, D], F32, kind="ExternalOutput")

    Exp = mybir.ActivationFunctionType.Exp

    with tile.TileContext(nc) as tc, ExitStack() as ctx:
        wpool = ctx.enter_context(tc.tile_pool(name="weights", bufs=1))
        spool = ctx.enter_context(tc.tile_pool(name="state", bufs=1))
        ptpool = ctx.enter_context(tc.tile_pool(name="pt", bufs=4))
        dpool = ctx.enter_context(tc.tile_pool(name="den", bufs=4))
        rpool = ctx.enter_context(tc.tile_pool(name="rec", bufs=8))
        scpool = ctx.enter_context(tc.tile_pool(name="scale", bufs=6))
        evpool = ctx.enter_context(tc.tile_pool(name="evac", bufs=4))
        ps_sc = ctx.enter_context(tc.tile_pool(name="pssc", bufs=2, space="PSUM"))
        ps_sm = ctx.enter_context(tc.tile_pool(name="pssm", bufs=4, space="PSUM"))

        # ---- persistent SBUF state (loaded once) ----
        xT_sb = wpool.tile([P, 8, S], BF16)
        xT_r = xT.rearrange("(o p) t -> p o t", p=P)
        for kk in range(8):  # split per chunk so early chunks land early
            nc.sync.dma_start(xT_sb[:, kk, :], xT_r[:, kk, :])
        wqT_sb = wpool.tile([P, 8, HHALF], BF16)
        nc.sync.dma_start(wqT_sb[:], wqT.rearrange("(o p) f -> p o f", p=P))
        wkT_sb = wpool.tile([P, 8, HHALF], BF16)
        nc.sync.dma_start(wkT_sb[:], wkT.rearrange("(o p) f -> p o f", p=P))
        wvT_sb = wpool.tile([P, 8, HHALF], BF16)
        nc.sync.dma_start(wvT_sb[:], wvT.rearrange("(o p) f -> p o f", p=P))
        dwT_sb = wpool.tile([P, 4, D], BF16)
        nc.sync.dma_start(dwT_sb[:], dwT.rearrange("(o p) f -> p o f", p=P))
        qb_sb = wpool.tile([P, 4], F32)
        nc.sync.dma_start(qb_sb[:], qb[:])
        kb_sb = wpool.tile([P, 4], F32)
        nc.sync.dma_start(kb_sb[:], kb[:])
        vb_sb = wpool.tile([P, 8, 64], BF16)
        nc.sync.dma_start(vb_sb[:], vb[:])

        # per-(pair,t4) projection tiles; per-strip vT tiles (fine-grained
        # tiles keep the scheduler's dependency tracking precise)
        q_sb = [[spool.tile([P, 512], BF16, name=f"q_{p}_{t}") for t in range(4)] for p in range(4)]
        k_sb = [[spool.tile([P, 512], BF16, name=f"k_{p}_{t}") for t in range(4)] for p in range(4)]
        # vT strip: 8 heads x (64 feats + ones col). memset once: the ones
        # columns (index 64) are never overwritten by the projection.
        vT_sb = [spool.tile([P, 8, 65], PT_DT, name=f"vT_{t}") for t in range(16)]
        for t in range(16):
            nc.vector.memset(vT_sb[t][:], 1.0)
        attU_sb = [spool.tile([P, 4, 512], BF16, name=f"attU_{q}") for q in range(4)]  # per qtc

        def v_unit(t):
            psv = ps_sm.tile([P, 8, 64], F32, tag="sm")
            for kk in range(8):
                nc.tensor.matmul(
                    psv[:],
                    lhsT=xT_sb[:, kk, t * 128 : (t + 1) * 128],
                    rhs=wvT_sb[:, kk, :],
                    start=(kk == 0),
                    stop=(kk == 7),
                )
            nc.vector.tensor_add(vT_sb[t][:, :, 0:64], psv[:], vb_sb[:])

        proj_state = {}

        def proj_q(p, j, q):
            """2 matmuls (quarter q) of projection unit j for pair p."""
            t4 = j // 2
            isq = j % 2 == 0
            w_sb = wqT_sb if isq else wkT_sb
            if q == 0:
                proj_state[(p, j)] = ps_sm.tile(
                    [P, 512], F32, tag="sm", name=f"psproj_{p}_{j}"
                )
            ps = proj_state[(p, j)]
            for kk in (2 * q, 2 * q + 1):
                nc.tensor.matmul(
                    ps[:],
                    lhsT=w_sb[:, kk, p * 128 : (p + 1) * 128],
                    rhs=xT_sb[:, kk, t4 * 512 : (t4 + 1) * 512],
                    start=(kk == 0),
                    stop=(kk == 7),
                )
            if q == 3:
                dst = q_sb[p][t4] if isq else k_sb[p][t4]
                b_sb = qb_sb if isq else kb_sb
                nc.vector.tensor_scalar_add(dst[:], ps[:], b_sb[:, p : p + 1])
                del proj_state[(p, j)]

        def proj_unit(p, j):
            for q in range(4):
                proj_q(p, j, q)

        dense_state = {}

        def dense_q(tt, oc, q):
            """2 matmuls (half q) of dense tile (tt, oc); evac+DMA on q==1."""
            qtc, ts = tt // 4, (tt % 4) * 128
            if q == 0:
                dense_state[(tt, oc)] = ps_sm.tile(
                    [P, 512], F32, tag="sm", name=f"psd_{tt}_{oc}"
                )
            ps = dense_state[(tt, oc)]
            for kk in (2 * q, 2 * q + 1):
                nc.tensor.matmul(
                    ps[:],
                    lhsT=attU_sb[qtc][:, kk, ts : ts + 128],
                    rhs=dwT_sb[:, kk, oc * 512 : (oc + 1) * 512],
                    start=(kk == 0),
                    stop=(kk == 3),
                )
            if q == 1:
                ot = evpool.tile([P, 512], F32, tag="out")
                nc.vector.tensor_copy(ot[:], ps[:])
                nc.sync.dma_start(
                    outp[tt * 128 : (tt + 1) * 128, oc * 512 : (oc + 1) * 512],
                    ot[:],
                )
                del dense_state[(tt, oc)]

        def body():
            # minimal startup prefix: q/k for (pair 0, first token quarter)
            # and V strip 0; the rest is produced just in time inside the
            # first attention block.
            for j in (0, 1):
                proj_unit(0, j)
            v_unit(0)

            for p in range(4):
                # side-work consumed one closure per odd kt iteration
                side = []
                if p == 0:
                    for j in (4, 6):  # q units for t4=2,3 (t4=1 in blk(0,0))
                        for q in range(4):
                            side.append(lambda j=j, q=q: proj_q(0, j, q))
                if p < 3:
                    for j in range(8):
                        for q in range(4):
                            side.append(lambda p=p, j=j, q=q: proj_q(p + 1, j, q))

                for qtc in range(4):
                    if p == 3 and qtc > 0:
                        for tt in range(4 * (qtc - 1), 4 * qtc):
                            for oc in range(2):
                                for q in range(2):
                                    side.append(
                                        lambda tt=tt, oc=oc, q=q: dense_q(tt, oc, q)
                                    )
                    qt = slice(qtc * 512, (qtc + 1) * 512)
                    ps_aA = ps_sm.tile([P, 512], F32, tag="sm")
                    ps_aB = ps_sm.tile([P, 512], F32, tag="sm")

                    def scores(kt):
                        sc = ps_sc.tile([P, 1024], F32, tag="sc")
                        kts = slice((kt % 4) * 128, (kt % 4) * 128 + 128)
                        nc.tensor.matmul(
                            sc[:, 0:512],
                            lhsT=k_sb[p][kt // 4][0:64, kts],
                            rhs=q_sb[p][qtc][0:64, :],
                            start=True,
                            stop=True,
                        )
                        nc.tensor.matmul(
                            sc[:, 512:1024],
                            lhsT=k_sb[p][kt // 4][64:128, kts],
                            rhs=q_sb[p][qtc][64:128, :],
                            start=True,
                            stop=True,
                        )
                        return sc

                    sc_cur = scores(0)
                    for kt in range(16):
                        if p == 0 and qtc == 0:
                            if kt < 15:
                                v_unit(kt + 1)  # strip kt+1 ready before its use
                            if kt in (1, 5, 9):  # k units t4=1..3, just in time
                                proj_unit(0, {1: 3, 5: 5, 9: 7}[kt])
                            elif kt in (11, 13):  # q unit t4=1 before blk(0,1)
                                proj_q(0, 2, 2 * (kt == 13))
                                proj_q(0, 2, 2 * (kt == 13) + 1)
                        elif side and (p < 3 or kt >= 4):
                            side.pop(0)()
                        pt = ptpool.tile([P, 1024], PT_DT, tag="pt")
                        nc.scalar.activation(pt[:], sc_cur[:], Exp, scale=0.125)
                        if kt < 15:
                            sc_cur = scores(kt + 1)
                        nc.tensor.matmul(
                            ps_aA[0:65, :],
                            lhsT=vT_sb[kt][:, 2 * p, :],
                            rhs=pt[:, 0:512],
                            start=(kt == 0),
                            stop=(kt == 15),
                        )
                        nc.tensor.matmul(
                            ps_aB[0:65, :],
                            lhsT=vT_sb[kt][:, 2 * p + 1, :],
                            rhs=pt[:, 512:1024],
                            start=(kt == 0),
                            stop=(kt == 15),
                        )

                    # Evacuate attended + denominator partition-aligned (the
                    # DVE has no cross-lane path: partition moves must go via
                    # SBUF->SBUF DMA), then normalize.
                    stA = dpool.tile([65, 512], F32, tag="d")
                    stB = dpool.tile([65, 512], F32, tag="d")
                    nc.vector.tensor_copy(stA[:], ps_aA[0:65, :])
                    nc.vector.tensor_copy(stB[:], ps_aB[0:65, :])
                    dA = rpool.tile([1, 512], F32, tag="r")
                    dB = rpool.tile([1, 512], F32, tag="r")
                    nc.sync.dma_start(dA[:], stA[64:65, :])
                    nc.sync.dma_start(dB[:], stB[64:65, :])
                    rA = rpool.tile([1, 512], F32, tag="r")
                    rB = rpool.tile([1, 512], F32, tag="r")
                    nc.vector.reciprocal_approx_fast(rA[:], dA[:])
                    nc.vector.reciprocal_approx_fast(rB[:], dB[:])
                    scA = scpool.tile([64, 512], F32, tag="s")
                    scB = scpool.tile([64, 512], F32, tag="s")
                    nc.gpsimd.partition_broadcast(scA[:, :], rA[0:1, :], 64)
                    nc.gpsimd.partition_broadcast(scB[:, :], rB[0:1, :], 64)
                    nc.vector.tensor_mul(
                        attU_sb[qtc][0:64, p, :], stA[0:64, :], scA[:, :]
                    )
                    attBn = scpool.tile([64, 512], BF16, tag="s")
                    nc.vector.tensor_mul(attBn[:, :], stB[0:64, :], scB[:, :])
                    nc.sync.dma_start(attU_sb[qtc][64:128, p, :], attBn[:, :])

                # flush any leftover side work before this pair's last block
                while side:
                    side.pop(0)()

            # dense tail: last token quarter
            for tt in range(12, 16):
                for oc in range(2):
                    dense_q(tt, oc, 0)
                    dense_q(tt, oc, 1)

        if loop_r:
            with tc.For_i(0, loop_r, 1):
                body()
        else:
            body()

    nc.compile()
    return nc


# ---------------------------------------------------------------------------
# PJRT runner (modeled on concourse.bass2jax.run_bass_via_pjrt, but caches the
# jitted executable so repeated calls don't retrace/recompile).
# ---------------------------------------------------------------------------
_CACHE = {}


def _make_runner(loop_r=None):
    import jax
    from jax.sharding import Mesh, PartitionSpec
    from jax.experimental.shard_map import shard_map

    from concourse import bass2jax
    from concourse import mybir as _mybir

    nc = _build_nc(loop_r=loop_r)
    bass2jax.install_neuronx_cc_hook()

    partition_name = nc.partition_id_tensor.name if nc.partition_id_tensor else None
    in_names, out_names, out_avals = [], [], []
    for alloc in nc.m.functions[0].allocations:
        if not isinstance(alloc, _mybir.MemoryLocationSet):
            continue
        name = alloc.memorylocations[0].name
        if alloc.kind == "ExternalInput":
            if name != partition_name:
                in_names.append(name)
        elif alloc.kind == "ExternalOutput":
            out_names.append(name)
            out_avals.append(
                jax.core.ShapedArray(
                    tuple(alloc.tensor_shape), _mybir.dt.np(alloc.dtype)
                )
            )
    n_params = len(in_names)
    all_in_names = list(in_names) + list(out_names)
    if partition_name is not None:
        all_in_names.append(partition_name)

    def _body(*args):
        operands = list(args)
        if partition_name is not None:
            operands.append(bass2jax.partition_id_tensor())
        outs = bass2jax._bass_exec_p.bind(
            *operands,
            out_avals=tuple(out_avals),
            in_names=tuple(all_in_names),
            out_names=tuple(out_names),
            lowering_input_output_aliases=(),
            sim_require_finite=True,
            sim_require_nnan=True,
            nc=nc,
        )
        return tuple(outs)

    devices = jax.devices()[:8]
    mesh = Mesh(np.asarray(devices), ("core",))
    in_specs = (PartitionSpec("core"),) * (n_params + len(out_names))
    out_specs = (PartitionSpec("core"),) * len(out_names)
    jitted = jax.jit(
        shard_map(
            _body, mesh=mesh, in_specs=in_specs, out_specs=out_specs, check_rep=False
        ),
        keep_unused=True,
    )
    zeros = [np.zeros((8 * av.shape[0], *av.shape[1:]), av.dtype) for av in out_avals]
    return (jitted, in_names, out_names, out_avals, zeros, mesh)


def _get_runner(loop_r=None):
    key = ("runner", loop_r)
    if key not in _CACHE:
        _CACHE[key] = _make_runner(loop_r)
    return _CACHE[key]


def _prep_core_inputs(x, wq_w, wq_b, wk_w, wk_b, wv_w, wv_b, dense_w):
    """Per-core host-side shard prep. Returns list of dicts (8 cores)."""
    maps = []
    for c in range(8):
        b, half = c // 2, c % 2
        f0 = half * HHALF
        fs = slice(f0, f0 + HHALF)
        maps.append(
            {
                "xT": np.ascontiguousarray(x[b].T).astype(NPBF16),
                "wqT": np.ascontiguousarray(wq_w[fs].T).astype(NPBF16),
                "wkT": np.ascontiguousarray(wk_w[fs].T).astype(NPBF16),
                "wvT": np.ascontiguousarray(wv_w[fs].T).astype(NPBF16),
                "dwT": np.ascontiguousarray(dense_w[:, fs].T).astype(NPBF16),
                "qb": np.ascontiguousarray(wq_b[fs].reshape(4, P).T.astype(np.float32)),
                "kb": np.ascontiguousarray(wk_b[fs].reshape(4, P).T.astype(np.float32)),
                "vb": np.broadcast_to(
                    wv_b[fs].reshape(1, 8, 64).astype(NPBF16), (P, 8, 64)
                ).copy(),
            }
        )
    return maps


def run_device(in_maps, time_iters=0, loop_r=None):
    """Run the SPMD kernel. Returns (per-core outp list, best wall ns or None)."""
    jitted, in_names, out_names, out_avals, zeros, mesh = _get_runner(loop_r)
    concat_in = [
        np.concatenate([in_maps[c][name] for c in range(8)], axis=0)
        for name in in_names
    ]
    args = concat_in + zeros
    outs = jitted(*args)
    outs = [np.asarray(o) for o in outs]
    best_ns = None
    if time_iters:
        import jax
        from jax.sharding import NamedSharding, PartitionSpec

        sh = NamedSharding(mesh, PartitionSpec("core"))
        dev_args = [jax.device_put(a, sh) for a in args]
        jax.block_until_ready(dev_args)
        times = []
        for _ in range(time_iters):
            t0 = time.perf_counter()
            o = jitted(*dev_args)
            jax.block_until_ready(o)
            times.append(time.perf_counter() - t0)
        best_ns = int(min(times) * 1e9)
    per_core = [
        {
            name: outs[i].reshape(8, *out_avals[i].shape)[c]
            for i, name in enumerate(out_names)
        }
        for c in range(8)
    ]
    return per_core, best_ns


def kernel(**inputs):
    x = np.asarray(inputs["x"], np.float32)
    args = {
        k: np.asarray(inputs[k], np.float32)
        for k in ["wq_w", "wq_b", "wk_w", "wk_b", "wv_w", "wv_b", "dense_w"]
    }
    in_maps = _prep_core_inputs(x, **args)
    per_core, _ = run_device(in_maps)
    dense_b = np.asarray(inputs["dense_b"], np.float32)
    out = np.empty((B, S, D), np.float32)
    for b in range(B):
        out[b] = per_core[2 * b]["outp"] + per_core[2 * b + 1]["outp"] + dense_b
    return out


# revision 13
# speedup vs baseline: 1.2505x; 1.2505x over previous
"""Trainium2 Bass kernel for MemoryEfficientMultiHeadAttention (8 NeuronCores).

Sharding: hybrid data/tensor parallel. Core c handles batch b = c//2 and head
group half = c%2 (8 of 16 heads, i.e. 512 of 1024 qkv features). Each core:
  q,k  = (x_b @ w.T + b) in [feat, tok] layout (feat on partitions)
  vT   = (x_b @ wv.T + b) in [tok, feat] layout, with a ones column appended
         per head so the attended matmul also produces the softmax denominator
  per head pair: scoresT = k_h.T @ q_h   (transposed scores, [kt, qt]; the two
                 heads run as row-tiled concurrent matmuls)
            PT = exp(scoresT / 8)        (no max-subtraction: scores are O(1))
            attU[0:64] / denom[64] += [vT_h | 1].T @ PT   (M=65, over kt tiles)
  attS = attU * (1/denom)  broadcast to partitions via gpsimd
  outp = attS.T @ dense_w_slice.T         (partial over this core's 512 feats)
Host: out[b] = outp[2b] + outp[2b+1] + dense_b.

The schedule targets the ACT engine's exp throughput (the measured roofline
for this decomposition): scores for iteration kt+1 are issued before the
attended matmuls of iteration kt so the PE never head-blocks the exp stream,
and the q/k projections of the next head pair plus the dense output matmuls
are drip-fed into the PE's idle slots (~2-matmul chunks, one per iteration).

All matmuls run in bf16 (1 cycle/row on TRN2 PE; fp32 is 4 cycles/row) with
fp32 PSUM accumulation.
"""

import sys
import time
from contextlib import ExitStack

import numpy as np

try:
    import concourse.bass as bass  # noqa: F401
except ImportError:  # pragma: no cover
    sys.path.insert(0, "/opt/trn_rl_repo")

import ml_dtypes

import concourse.bacc as bacc
import concourse.mybir as mybir
import concourse.tile as tile

P = 128
BF16 = mybir.dt.bfloat16
FP8 = mybir.dt.float8e4
F32 = mybir.dt.float32
NPBF16 = ml_dtypes.bfloat16

# exp output / V dtype. fp8e4m3 would halve ACT SBUF-write and PE rhs-read
# traffic, but measures ~2e-2 rel err (attention outputs are themselves
# softmax averages, so quantization noise is not averaged down) — keep bf16.
PT_DT = BF16

B, S, D = 4, 2048, 1024
HHALF = 512  # features per core (8 heads x 64)


def _build_nc(loop_r=None):
    nc = bacc.Bacc()

    xT = nc.dram_tensor("xT", [D, S], BF16, kind="ExternalInput")
    wqT = nc.dram_tensor("wqT", [D, HHALF], BF16, kind="ExternalInput")
    wkT = nc.dram_tensor("wkT", [D, HHALF], BF16, kind="ExternalInput")
    wvT = nc.dram_tensor("wvT", [D, HHALF], BF16, kind="ExternalInput")
    dwT = nc.dram_tensor("dwT", [HHALF, D], BF16, kind="ExternalInput")
    qb = nc.dram_tensor("qb", [P, 4], F32, kind="ExternalInput")
    kb = nc.dram_tensor("kb", [P, 4], F32, kind="ExternalInput")
    vb = nc.dram_tensor("vb", [P, 8, 64], BF16, kind="ExternalInput")
    outp = nc.dram_tensor("outp", [S, D], F32, kind="ExternalOutput")

    Exp = mybir.ActivationFunctionType.Exp

    with tile.TileContext(nc) as tc, ExitStack() as ctx:
        wpool = ctx.enter_context(tc.tile_pool(name="weights", bufs=1))
        spool = ctx.enter_context(tc.tile_pool(name="state", bufs=1))
        ptpool = ctx.enter_context(tc.tile_pool(name="pt", bufs=4))
        dpool = ctx.enter_context(tc.tile_pool(name="den", bufs=4))
        rpool = ctx.enter_context(tc.tile_pool(name="rec", bufs=8))
        scpool = ctx.enter_context(tc.tile_pool(name="scale", bufs=6))
        evpool = ctx.enter_context(tc.tile_pool(name="evac", bufs=4))
        ps_sc = ctx.enter_context(tc.tile_pool(name="pssc", bufs=2, space="PSUM"))
        ps_sm = ctx.enter_context(tc.tile_pool(name="pssm", bufs=4, space="PSUM"))

        # ---- persistent SBUF state (loaded once) ----
        xT_sb = wpool.tile([P, 8, S], BF16)
        xT_r = xT.rearrange("(o p) t -> p o t", p=P)
        for kk in range(8):  # split per chunk so early chunks land early
            nc.sync.dma_start(xT_sb[:, kk, :], xT_r[:, kk, :])
        wqT_sb = wpool.tile([P, 8, HHALF], BF16)
        nc.sync.dma_start(wqT_sb[:], wqT.rearrange("(o p) f -> p o f", p=P))
        wkT_sb = wpool.tile([P, 8, HHALF], BF16)
        nc.sync.dma_start(wkT_sb[:], wkT.rearrange("(o p) f -> p o f", p=P))
        wvT_sb = wpool.tile([P, 8, HHALF], BF16)
        nc.sync.dma_start(wvT_sb[:], wvT.rearrange("(o p) f -> p o f", p=P))
        dwT_sb = wpool.tile([P, 4, D], BF16)
        nc.sync.dma_start(dwT_sb[:], dwT.rearrange("(o p) f -> p o f", p=P))
        qb_sb = wpool.tile([P, 4], F32)
        nc.sync.dma_start(qb_sb[:], qb[:])
        kb_sb = wpool.tile([P, 4], F32)
        nc.sync.dma_start(kb_sb[:], kb[:])
        vb_sb = wpool.tile([P, 8, 64], BF16)
        nc.sync.dma_start(vb_sb[:], vb[:])

        # per-(pair,t4) projection tiles; per-strip vT tiles (fine-grained
        # tiles keep the scheduler's dependency tracking precise)
        q_sb = [[spool.tile([P, 512], BF16, name=f"q_{p}_{t}") for t in range(4)] for p in range(4)]
        k_sb = [[spool.tile([P, 512], BF16, name=f"k_{p}_{t}") for t in range(4)] for p in range(4)]
        # vT strip: 8 heads x (64 feats + ones col). memset once: the ones
        # columns (index 64) are never overwritten by the projection.
        vT_sb = [spool.tile([P, 8, 65], PT_DT, name=f"vT_{t}") for t in range(16)]
        for t in range(16):
            nc.vector.memset(vT_sb[t][:], 1.0)
        attU_sb = [spool.tile([P, 4, 512], BF16, name=f"attU_{q}") for q in range(4)]  # per qtc

        def v_unit(t):
            psv = ps_sm.tile([P, 8, 64], F32, tag="sm")
            for kk in range(8):
                nc.tensor.matmul(
                    psv[:],
                    lhsT=xT_sb[:, kk, t * 128 : (t + 1) * 128],
                    rhs=wvT_sb[:, kk, :],
                    start=(kk == 0),
                    stop=(kk == 7),
                )
            nc.vector.tensor_add(vT_sb[t][:, :, 0:64], psv[:], vb_sb[:])

        proj_state = {}

        def proj_q(p, j, q):
            """2 matmuls (quarter q) of projection unit j for pair p."""
            t4 = j // 2
            isq = j % 2 == 0
            w_sb = wqT_sb if isq else wkT_sb
            if q == 0:
                proj_state[(p, j)] = ps_sm.tile(
                    [P, 512], F32, tag="sm", name=f"psproj_{p}_{j}"
                )
            ps = proj_state[(p, j)]
            for kk in (2 * q, 2 * q + 1):
                nc.tensor.matmul(
                    ps[:],
                    lhsT=w_sb[:, kk, p * 128 : (p + 1) * 128],
                    rhs=xT_sb[:, kk, t4 * 512 : (t4 + 1) * 512],
                    start=(kk == 0),
                    stop=(kk == 7),
                )
            if q == 3:
                dst = q_sb[p][t4] if isq else k_sb[p][t4]
                b_sb = qb_sb if isq else kb_sb
                nc.vector.tensor_scalar_add(dst[:], ps[:], b_sb[:, p : p + 1])
                del proj_state[(p, j)]

        def proj_unit(p, j):
            for q in range(4):
                proj_q(p, j, q)

        dense_state = {}

        def dense_q(tt, oc, q):
            """2 matmuls (half q) of dense tile (tt, oc); evac+DMA on q==1."""
            qtc, ts = tt // 4, (tt % 4) * 128
            if q == 0:
                dense_state[(tt, oc)] = ps_sm.tile(
                    [P, 512], F32, tag="sm", name=f"psd_{tt}_{oc}"
                )
            ps = dense_state[(tt, oc)]
            for kk in (2 * q, 2 * q + 1):
                nc.tensor.matmul(
                    ps[:],
                    lhsT=attU_sb[qtc][:, kk, ts : ts + 128],
                    rhs=dwT_sb[:, kk, oc * 512 : (oc + 1) * 512],
                    start=(kk == 0),
                    stop=(kk == 3),
                )
            if q == 1:
                ot = evpool.tile([P, 512], F32, tag="out")
                nc.vector.tensor_copy(ot[:], ps[:])
                nc.sync.dma_start(
                    outp[tt * 128 : (tt + 1) * 128, oc * 512 : (oc + 1) * 512],
                    ot[:],
                )
                del dense_state[(tt, oc)]

        def body():
            # minimal startup prefix: q/k for (pair 0, first token quarter)
            # and V strip 0; the rest is produced just in time inside the
            # first attention block.
            for j in (0, 1):
                proj_unit(0, j)
            v_unit(0)

            for p in range(4):
                # side-work consumed one closure per odd kt iteration
                side = []
                if p == 0:
                    for j in (4, 6):  # q units for t4=2,3 (t4=1 in blk(0,0))
                        for q in range(4):
                            side.append(lambda j=j, q=q: proj_q(0, j, q))
                if p < 3:
                    for j in range(8):
                        for q in range(4):
                            side.append(lambda p=p, j=j, q=q: proj_q(p + 1, j, q))

                for qtc in range(4):
                    if p == 3 and qtc > 0:
                        for tt in range(4 * (qtc - 1), 4 * qtc):
                            for oc in range(2):
                                for q in range(2):
                                    side.append(
                                        lambda tt=tt, oc=oc, q=q: dense_q(tt, oc, q)
                                    )
                    qt = slice(qtc * 512, (qtc + 1) * 512)
                    ps_aA = ps_sm.tile([P, 512], F32, tag="sm")
                    ps_aB = ps_sm.tile([P, 512], F32, tag="sm")

                    def scores(kt):
                        sc = ps_sc.tile([P, 1024], F32, tag="sc")
                        kts = slice((kt % 4) * 128, (kt % 4) * 128 + 128)
                        nc.tensor.matmul(
                            sc[:, 0:512],
                            lhsT=k_sb[p][kt // 4][0:64, kts],
                            rhs=q_sb[p][qtc][0:64, :],
                            start=True,
                            stop=True,
                        )
                        nc.tensor.matmul(
                            sc[:, 512:1024],
                            lhsT=k_sb[p][kt // 4][64:128, kts],
                            rhs=q_sb[p][qtc][64:128, :],
                            start=True,
                            stop=True,
                        )
                        return sc

                    sc_cur = scores(0)
                    for kt in range(16):
                        if p == 0 and qtc == 0:
                            if kt < 15:
                                v_unit(kt + 1)  # strip kt+1 ready before its use
                            if kt in (1, 5, 9):  # k units t4=1..3, just in time
                                proj_unit(0, {1: 3, 5: 5, 9: 7}[kt])
                            elif kt in (11, 13):  # q unit t4=1 before blk(0,1)
                                proj_q(0, 2, 2 * (kt == 13))
                                proj_q(0, 2, 2 * (kt == 13) + 1)
                        elif side and (p < 3 or kt >= 4):
                            side.pop(0)()
                        pt = ptpool.tile([P, 1024], PT_DT, tag="pt")
                        nc.scalar.activation(pt[:], sc_cur[:], Exp, scale=0.125)
                        if kt < 15:
                            sc_cur = scores(kt + 1)
                        nc.tensor.matmul(
                            ps_aA[0:65, :],
                            lhsT=vT_sb[kt][:, 2 * p, :],
                            rhs=pt[:, 0:512],
                            start=(kt == 0),
                            stop=(kt == 15),
                        )
                        nc.tensor.matmul(
                            ps_aB[0:65, :],
                            lhsT=vT_sb[kt][:, 2 * p + 1, :],
                            rhs=pt[:, 512:1024],
                            start=(kt == 0),
                            stop=(kt == 15),
                        )

                    # Evacuate attended + denominator partition-aligned (the
                    # DVE has no cross-lane path: partition moves must go via
                    # SBUF->SBUF DMA), then normalize.
                    stA = dpool.tile([65, 512], F32, tag="d")
                    stB = dpool.tile([65, 512], F32, tag="d")
                    nc.vector.tensor_copy(stA[:], ps_aA[0:65, :])
                    nc.vector.tensor_copy(stB[:], ps_aB[0:65, :])
                    dA = rpool.tile([1, 512], F32, tag="r")
                    dB = rpool.tile([1, 512], F32, tag="r")
                    nc.sync.dma_start(dA[:], stA[64:65, :])
                    nc.sync.dma_start(dB[:], stB[64:65, :])
                    rA = rpool.tile([1, 512], F32, tag="r")
                    rB = rpool.tile([1, 512], F32, tag="r")
                    nc.vector.reciprocal_approx_fast(rA[:], dA[:])
                    nc.vector.reciprocal_approx_fast(rB[:], dB[:])
                    scA = scpool.tile([64, 512], F32, tag="s")
                    scB = scpool.tile([64, 512], F32, tag="s")
                    nc.gpsimd.partition_broadcast(scA[:, :], rA[0:1, :], 64)
                    nc.gpsimd.partition_broadcast(scB[:, :], rB[0:1, :], 64)
                    nc.vector.tensor_mul(
                        attU_sb[qtc][0:64, p, :], stA[0:64, :], scA[:, :]
                    )
                    attBn = scpool.tile([64, 512], BF16, tag="s")
                    nc.vector.tensor_mul(attBn[:, :], stB[0:64, :], scB[:, :])
                    nc.sync.dma_start(attU_sb[qtc][64:128, p, :], attBn[:, :])

                # flush any leftover side work before this pair's last block
                while side:
                    side.pop(0)()

            # dense tail: last token quarter
            for tt in range(12, 16):
                for oc in range(2):
                    dense_q(tt, oc, 0)
                    dense_q(tt, oc, 1)

        if loop_r:
            with tc.For_i(0, loop_r, 1):
                body()
        else:
            body()

    nc.compile()
    return nc


# ---------------------------------------------------------------------------
# PJRT runner (modeled on concourse.bass2jax.run_bass_via_pjrt, but caches the
# jitted executable so repeated calls don't retrace/recompile).
# ---------------------------------------------------------------------------
_CACHE = {}


def _make_runner(loop_r=None):
    import jax
    from jax.sharding import Mesh, PartitionSpec
    from jax.experimental.shard_map import shard_map

    from concourse import bass2jax
    from concourse import mybir as _mybir

    nc = _build_nc(loop_r=loop_r)
    bass2jax.install_neuronx_cc_hook()

    partition_name = nc.partition_id_tensor.name if nc.partition_id_tensor else None
    in_names, out_names, out_avals = [], [], []
    for alloc in nc.m.functions[0].allocations:
        if not isinstance(alloc, _mybir.MemoryLocationSet):
            continue
        name = alloc.memorylocations[0].name
        if alloc.kind == "ExternalInput":
            if name != partition_name:
                in_names.append(name)
        elif alloc.kind == "ExternalOutput":
            out_names.append(name)
            out_avals.append(
                jax.core.ShapedArray(
                    tuple(alloc.tensor_shape), _mybir.dt.np(alloc.dtype)
                )
            )
    n_params = len(in_names)
    all_in_names = list(in_names) + list(out_names)
    if partition_name is not None:
        all_in_names.append(partition_name)

    def _body(*args):
        operands = list(args)
        if partition_name is not None:
            operands.append(bass2jax.partition_id_tensor())
        outs = bass2jax._bass_exec_p.bind(
            *operands,
            out_avals=tuple(out_avals),
            in_names=tuple(all_in_names),
            out_names=tuple(out_names),
            lowering_input_output_aliases=(),
            sim_require_finite=True,
            sim_require_nnan=True,
            nc=nc,
        )
        return tuple(outs)

    devices = jax.devices()[:8]
    mesh = Mesh(np.asarray(devices), ("core",))
    in_specs = (PartitionSpec("core"),) * (n_params + len(out_names))
    out_specs = (PartitionSpec("core"),) * len(out_names)
    jitted = jax.jit(
        shard_map(
            _body, mesh=mesh, in_specs=in_specs, out_specs=out_specs, check_rep=False
        ),
        keep_unused=True,
    )
    zeros = [np.zeros((8 * av.shape[0], *av.shape[1:]), av.dtype) for av in out_avals]
    return (jitted, in_names, out_names, out_avals, zeros, mesh)


def _get_runner(loop_r=None):
    key = ("runner", loop_r)
    if key not in _CACHE:
        _CACHE[key] = _make_runner(loop_r)
    return _CACHE[key]


def _prep_core_inputs(x, wq_w, wq_b, wk_w, wk_b, wv_w, wv_b, dense_w):
    """Per-core host-side shard prep. Returns list of dicts (8 cores)."""
    maps = []
    for c in range(8):
        b, half = c // 2, c % 2
        f0 = half * HHALF
        fs = slice(f0, f0 + HHALF)
        maps.append(
            {
                "xT": np.ascontiguousarray(x[b].T).astype(NPBF16),
                "wqT": np.ascontiguousarray(wq_w[fs].T).astype(NPBF16),
                "wkT": np.ascontiguousarray(wk_w[fs].T).astype(NPBF16),
                "wvT": np.ascontiguousarray(wv_w[fs].T).astype(NPBF16),
                "dwT": np.ascontiguousarray(dense_w[:, fs].T).astype(NPBF16),
                "qb": np.ascontiguousarray(wq_b[fs].reshape(4, P).T.astype(np.float32)),
                "kb": np.ascontiguousarray(wk_b[fs].reshape(4, P).T.astype(np.float32)),
                "vb": np.broadcast_to(
                    wv_b[fs].reshape(1, 8, 64).astype(NPBF16), (P, 8, 64)
                ).copy(),
            }
        )
    return maps


def run_device(in_maps, time_iters=0, loop_r=None):
    """Run the SPMD kernel. Returns (per-core outp list, best wall ns or None)."""
    jitted, in_names, out_names, out_avals, zeros, mesh = _get_runner(loop_r)
    concat_in = [
        np.concatenate([in_maps[c][name] for c in range(8)], axis=0)
        for name in in_names
    ]
    args = concat_in + zeros
    outs = jitted(*args)
    outs = [np.asarray(o) for o in outs]
    best_ns = None
    if time_iters:
        import jax
        from jax.sharding import NamedSharding, PartitionSpec

        sh = NamedSharding(mesh, PartitionSpec("core"))
        dev_args = [jax.device_put(a, sh) for a in args]
        jax.block_until_ready(dev_args)
        times = []
        for _ in range(time_iters):
            t0 = time.perf_counter()
            o = jitted(*dev_args)
            jax.block_until_ready(o)
            times.append(time.perf_counter() - t0)
        best_ns = int(min(times) * 1e9)
    per_core = [
        {
            name: outs[i].reshape(8, *out_avals[i].shape)[c]
            for i, name in enumerate(out_names)
        }
        for c in range(8)
    ]
    return per_core, best_ns


def kernel(**inputs):
    x = np.asarray(inputs["x"], np.float32)
    args = {
        k: np.asarray(inputs[k], np.float32)
        for k in ["wq_w", "wq_b", "wk_w", "wk_b", "wv_w", "wv_b", "dense_w"]
    }
    in_maps = _prep_core_inputs(x, **args)
    per_core, _ = run_device(in_maps)
    dense_b = np.asarray(inputs["dense_b"], np.float32)
    out = np.empty((B, S, D), np.float32)
    for b in range(B):
        out[b] = per_core[2 * b]["outp"] + per_core[2 * b + 1]["outp"] + dense_b
    return out


# revision 14
# speedup vs baseline: 1.2945x; 1.0352x over previous
"""Trainium2 Bass kernel for MemoryEfficientMultiHeadAttention (8 NeuronCores).

Sharding: hybrid data/tensor parallel. Core c handles batch b = c//2 and head
group half = c%2 (8 of 16 heads, i.e. 512 of 1024 qkv features). Each core:
  q,k  = (x_b @ w.T + b) in [feat, tok] layout (feat on partitions)
  vT   = (x_b @ wv.T + b) in [tok, feat] layout, with a ones column appended
         per head so the attended matmul also produces the softmax denominator
  per head pair: scoresT = k_h.T @ q_h   (transposed scores, [kt, qt]; the two
                 heads run as row-tiled concurrent matmuls)
            PT = exp(scoresT / 8)        (no max-subtraction: scores are O(1))
            attU[0:64] / denom[64] += [vT_h | 1].T @ PT   (M=65, over kt tiles)
  attS = attU * (1/denom)  broadcast to partitions via gpsimd
  outp = attS.T @ dense_w_slice.T         (partial over this core's 512 feats)
Host: out[b] = outp[2b] + outp[2b+1] + dense_b.

The schedule targets the ACT engine's exp throughput (the measured roofline
for this decomposition): scores for iteration kt+1 are issued before the
attended matmuls of iteration kt so the PE never head-blocks the exp stream,
and the q/k projections of the next head pair plus the dense output matmuls
are drip-fed into the PE's idle slots (~2-matmul chunks, one per iteration).

All matmuls run in bf16 (1 cycle/row on TRN2 PE; fp32 is 4 cycles/row) with
fp32 PSUM accumulation.
"""

import sys
import time
from contextlib import ExitStack

import numpy as np

try:
    import concourse.bass as bass  # noqa: F401
except ImportError:  # pragma: no cover
    sys.path.insert(0, "/opt/trn_rl_repo")

import ml_dtypes

import concourse.bacc as bacc
import concourse.mybir as mybir
import concourse.tile as tile

P = 128
BF16 = mybir.dt.bfloat16
FP8 = mybir.dt.float8e4
F32 = mybir.dt.float32
NPBF16 = ml_dtypes.bfloat16

# exp output / V dtype. fp8e4m3 would halve ACT SBUF-write and PE rhs-read
# traffic, but measures ~2e-2 rel err (attention outputs are themselves
# softmax averages, so quantization noise is not averaged down) — keep bf16.
PT_DT = BF16

B, S, D = 4, 2048, 1024
HHALF = 512  # features per core (8 heads x 64)


def _build_nc(loop_r=None):
    nc = bacc.Bacc()

    xT = nc.dram_tensor("xT", [D, S], BF16, kind="ExternalInput")
    wqT = nc.dram_tensor("wqT", [D, HHALF], BF16, kind="ExternalInput")
    wkT = nc.dram_tensor("wkT", [D, HHALF], BF16, kind="ExternalInput")
    wvT = nc.dram_tensor("wvT", [D, HHALF], BF16, kind="ExternalInput")
    dwT = nc.dram_tensor("dwT", [HHALF, D], BF16, kind="ExternalInput")
    qb = nc.dram_tensor("qb", [P, 4], F32, kind="ExternalInput")
    kb = nc.dram_tensor("kb", [P, 4], F32, kind="ExternalInput")
    vb = nc.dram_tensor("vb", [P, 8, 64], BF16, kind="ExternalInput")
    outp = nc.dram_tensor("outp", [S, D], F32, kind="ExternalOutput")

    Exp = mybir.ActivationFunctionType.Exp

    with tile.TileContext(nc) as tc, ExitStack() as ctx:
        wpool = ctx.enter_context(tc.tile_pool(name="weights", bufs=1))
        spool = ctx.enter_context(tc.tile_pool(name="state", bufs=1))
        ptpool = ctx.enter_context(tc.tile_pool(name="pt", bufs=4))
        dpool = ctx.enter_context(tc.tile_pool(name="den", bufs=4))
        rpool = ctx.enter_context(tc.tile_pool(name="rec", bufs=8))
        scpool = ctx.enter_context(tc.tile_pool(name="scale", bufs=6))
        evpool = ctx.enter_context(tc.tile_pool(name="evac", bufs=4))
        ps_sc = ctx.enter_context(tc.tile_pool(name="pssc", bufs=2, space="PSUM"))
        ps_sm = ctx.enter_context(tc.tile_pool(name="pssm", bufs=4, space="PSUM"))

        # ---- persistent SBUF state (loaded once) ----
        xT_sb = wpool.tile([P, 8, S], BF16)
        xT_r = xT.rearrange("(o p) t -> p o t", p=P)
        for kk in range(8):  # split per chunk so early chunks land early
            nc.sync.dma_start(xT_sb[:, kk, :], xT_r[:, kk, :])
        wqT_sb = wpool.tile([P, 8, HHALF], BF16)
        nc.sync.dma_start(wqT_sb[:], wqT.rearrange("(o p) f -> p o f", p=P))
        wkT_sb = wpool.tile([P, 8, HHALF], BF16)
        nc.sync.dma_start(wkT_sb[:], wkT.rearrange("(o p) f -> p o f", p=P))
        wvT_sb = wpool.tile([P, 8, HHALF], BF16)
        nc.sync.dma_start(wvT_sb[:], wvT.rearrange("(o p) f -> p o f", p=P))
        dwT_sb = wpool.tile([P, 4, D], BF16)
        nc.sync.dma_start(dwT_sb[:], dwT.rearrange("(o p) f -> p o f", p=P))
        qb_sb = wpool.tile([P, 4], F32)
        nc.sync.dma_start(qb_sb[:], qb[:])
        kb_sb = wpool.tile([P, 4], F32)
        nc.sync.dma_start(kb_sb[:], kb[:])
        vb_sb = wpool.tile([P, 8, 64], BF16)
        nc.sync.dma_start(vb_sb[:], vb[:])

        # per-(pair,t4) projection tiles; per-strip vT tiles (fine-grained
        # tiles keep the scheduler's dependency tracking precise)
        q_sb = [[spool.tile([P, 512], BF16, name=f"q_{p}_{t}") for t in range(4)] for p in range(4)]
        k_sb = [[spool.tile([P, 512], BF16, name=f"k_{p}_{t}") for t in range(4)] for p in range(4)]
        # vT strip: 8 heads x (64 feats + ones col). memset once: the ones
        # columns (index 64) are never overwritten by the projection.
        vT_sb = [spool.tile([P, 8, 65], PT_DT, name=f"vT_{t}") for t in range(16)]
        for t in range(16):
            nc.vector.memset(vT_sb[t][:], 1.0)
        attU_sb = [spool.tile([P, 4, 512], BF16, name=f"attU_{q}") for q in range(4)]  # per qtc

        def v_unit(t):
            psv = ps_sm.tile([P, 8, 64], F32, tag="sm")
            for kk in range(8):
                nc.tensor.matmul(
                    psv[:],
                    lhsT=xT_sb[:, kk, t * 128 : (t + 1) * 128],
                    rhs=wvT_sb[:, kk, :],
                    start=(kk == 0),
                    stop=(kk == 7),
                )
            nc.vector.tensor_add(vT_sb[t][:, :, 0:64], psv[:], vb_sb[:])

        proj_state = {}

        def proj_q(p, j, q):
            """2 matmuls (quarter q) of projection unit j for pair p."""
            t4 = j // 2
            isq = j % 2 == 0
            w_sb = wqT_sb if isq else wkT_sb
            if q == 0:
                proj_state[(p, j)] = ps_sm.tile(
                    [P, 512], F32, tag="sm", name=f"psproj_{p}_{j}"
                )
            ps = proj_state[(p, j)]
            for kk in (2 * q, 2 * q + 1):
                nc.tensor.matmul(
                    ps[:],
                    lhsT=w_sb[:, kk, p * 128 : (p + 1) * 128],
                    rhs=xT_sb[:, kk, t4 * 512 : (t4 + 1) * 512],
                    start=(kk == 0),
                    stop=(kk == 7),
                )
            if q == 3:
                dst = q_sb[p][t4] if isq else k_sb[p][t4]
                b_sb = qb_sb if isq else kb_sb
                nc.vector.tensor_scalar_add(dst[:], ps[:], b_sb[:, p : p + 1])
                del proj_state[(p, j)]

        def proj_unit(p, j):
            for q in range(4):
                proj_q(p, j, q)

        dense_state = {}

        def dense_q(tt, oc, q):
            """2 matmuls (half q) of dense tile (tt, oc); evac+DMA on q==1."""
            qtc, ts = tt // 4, (tt % 4) * 128
            if q == 0:
                dense_state[(tt, oc)] = ps_sm.tile(
                    [P, 512], F32, tag="sm", name=f"psd_{tt}_{oc}"
                )
            ps = dense_state[(tt, oc)]
            for kk in (2 * q, 2 * q + 1):
                nc.tensor.matmul(
                    ps[:],
                    lhsT=attU_sb[qtc][:, kk, ts : ts + 128],
                    rhs=dwT_sb[:, kk, oc * 512 : (oc + 1) * 512],
                    start=(kk == 0),
                    stop=(kk == 3),
                )
            if q == 1:
                ot = evpool.tile([P, 512], F32, tag="out")
                nc.vector.tensor_copy(ot[:], ps[:])
                nc.sync.dma_start(
                    outp[tt * 128 : (tt + 1) * 128, oc * 512 : (oc + 1) * 512],
                    ot[:],
                )
                del dense_state[(tt, oc)]

        def body():
            # minimal startup prefix: q/k for (pair 0, first token quarter)
            # and V strip 0; the rest is produced just in time inside the
            # first attention block.
            for j in (0, 1):
                proj_unit(0, j)
            v_unit(0)

            for p in range(4):
                # side-work consumed one closure per odd kt iteration
                side = []
                if p == 0:
                    for j in (4, 6):  # q units for t4=2,3 (t4=1 in blk(0,0))
                        for q in range(4):
                            side.append(lambda j=j, q=q: proj_q(0, j, q))
                if p < 3:
                    for j in range(8):
                        for q in range(4):
                            side.append(lambda p=p, j=j, q=q: proj_q(p + 1, j, q))

                for qtc in range(4):
                    if p == 3 and qtc > 0:
                        for tt in range(4 * (qtc - 1), 4 * qtc):
                            for oc in range(2):
                                for q in range(2):
                                    side.append(
                                        lambda tt=tt, oc=oc, q=q: dense_q(tt, oc, q)
                                    )
                    qt = slice(qtc * 512, (qtc + 1) * 512)
                    ps_aA = ps_sm.tile([P, 512], F32, tag="sm")
                    ps_aB = ps_sm.tile([P, 512], F32, tag="sm")

                    def scores(kt):
                        sc = ps_sc.tile([P, 1024], F32, tag="sc")
                        kts = slice((kt % 4) * 128, (kt % 4) * 128 + 128)
                        nc.tensor.matmul(
                            sc[:, 0:512],
                            lhsT=k_sb[p][kt // 4][0:64, kts],
                            rhs=q_sb[p][qtc][0:64, :],
                            start=True,
                            stop=True,
                        )
                        nc.tensor.matmul(
                            sc[:, 512:1024],
                            lhsT=k_sb[p][kt // 4][64:128, kts],
                            rhs=q_sb[p][qtc][64:128, :],
                            start=True,
                            stop=True,
                        )
                        return sc

                    sc_cur = scores(0)
                    for kt in range(16):
                        if p == 0 and qtc == 0:
                            if kt < 15:
                                v_unit(kt + 1)  # strip kt+1 ready before its use
                            if kt in (1, 5, 9):  # k units t4=1..3, just in time
                                proj_unit(0, {1: 3, 5: 5, 9: 7}[kt])
                            elif kt in (11, 13):  # q unit t4=1 before blk(0,1)
                                proj_q(0, 2, 2 * (kt == 13))
                                proj_q(0, 2, 2 * (kt == 13) + 1)
                        elif (
                            side
                            and (p < 3 or kt >= 2)
                            and (kt % 2 == 1 or len(side) > 16)
                        ):
                            # pace side work to stay under the exp cadence:
                            # every other iteration once the backlog is small
                            side.pop(0)()
                        pt = ptpool.tile([P, 1024], PT_DT, tag="pt")
                        nc.scalar.activation(pt[:], sc_cur[:], Exp, scale=0.125)
                        if kt < 15:
                            sc_cur = scores(kt + 1)
                        nc.tensor.matmul(
                            ps_aA[0:65, :],
                            lhsT=vT_sb[kt][:, 2 * p, :],
                            rhs=pt[:, 0:512],
                            start=(kt == 0),
                            stop=(kt == 15),
                        )
                        nc.tensor.matmul(
                            ps_aB[0:65, :],
                            lhsT=vT_sb[kt][:, 2 * p + 1, :],
                            rhs=pt[:, 512:1024],
                            start=(kt == 0),
                            stop=(kt == 15),
                        )

                    # Evacuate attended + denominator partition-aligned (the
                    # DVE has no cross-lane path: partition moves must go via
                    # SBUF->SBUF DMA), then normalize.
                    stA = dpool.tile([65, 512], F32, tag="d")
                    stB = dpool.tile([65, 512], F32, tag="d")
                    nc.vector.tensor_copy(stA[:], ps_aA[0:65, :])
                    nc.vector.tensor_copy(stB[:], ps_aB[0:65, :])
                    dA = rpool.tile([1, 512], F32, tag="r")
                    dB = rpool.tile([1, 512], F32, tag="r")
                    nc.sync.dma_start(dA[:], stA[64:65, :])
                    nc.sync.dma_start(dB[:], stB[64:65, :])
                    rA = rpool.tile([1, 512], F32, tag="r")
                    rB = rpool.tile([1, 512], F32, tag="r")
                    nc.vector.reciprocal_approx_fast(rA[:], dA[:])
                    nc.vector.reciprocal_approx_fast(rB[:], dB[:])
                    scA = scpool.tile([64, 512], F32, tag="s")
                    scB = scpool.tile([64, 512], F32, tag="s")
                    nc.gpsimd.partition_broadcast(scA[:, :], rA[0:1, :], 64)
                    nc.gpsimd.partition_broadcast(scB[:, :], rB[0:1, :], 64)
                    nc.vector.tensor_mul(
                        attU_sb[qtc][0:64, p, :], stA[0:64, :], scA[:, :]
                    )
                    attBn = scpool.tile([64, 512], BF16, tag="s")
                    nc.vector.tensor_mul(attBn[:, :], stB[0:64, :], scB[:, :])
                    nc.sync.dma_start(attU_sb[qtc][64:128, p, :], attBn[:, :])

                # flush any leftover side work before this pair's last block
                while side:
                    side.pop(0)()

            # dense tail: last token quarter
            for tt in range(12, 16):
                for oc in range(2):
                    dense_q(tt, oc, 0)
                    dense_q(tt, oc, 1)

        if loop_r:
            with tc.For_i(0, loop_r, 1):
                body()
        else:
            body()

    nc.compile()
    return nc


# ---------------------------------------------------------------------------
# PJRT runner (modeled on concourse.bass2jax.run_bass_via_pjrt, but caches the
# jitted executable so repeated calls don't retrace/recompile).
# ---------------------------------------------------------------------------
_CACHE = {}


def _make_runner(loop_r=None):
    import jax
    from jax.sharding import Mesh, PartitionSpec
    from jax.experimental.shard_map import shard_map

    from concourse import bass2jax
    from concourse import mybir as _mybir

    nc = _build_nc(loop_r=loop_r)
    bass2jax.install_neuronx_cc_hook()

    partition_name = nc.partition_id_tensor.name if nc.partition_id_tensor else None
    in_names, out_names, out_avals = [], [], []
    for alloc in nc.m.functions[0].allocations:
        if not isinstance(alloc, _mybir.MemoryLocationSet):
            continue
        name = alloc.memorylocations[0].name
        if alloc.kind == "ExternalInput":
            if name != partition_name:
                in_names.append(name)
        elif alloc.kind == "ExternalOutput":
            out_names.append(name)
            out_avals.append(
                jax.core.ShapedArray(
                    tuple(alloc.tensor_shape), _mybir.dt.np(alloc.dtype)
                )
            )
    n_params = len(in_names)
    all_in_names = list(in_names) + list(out_names)
    if partition_name is not None:
        all_in_names.append(partition_name)

    def _body(*args):
        operands = list(args)
        if partition_name is not None:
            operands.append(bass2jax.partition_id_tensor())
        outs = bass2jax._bass_exec_p.bind(
            *operands,
            out_avals=tuple(out_avals),
            in_names=tuple(all_in_names),
            out_names=tuple(out_names),
            lowering_input_output_aliases=(),
            sim_require_finite=True,
            sim_require_nnan=True,
            nc=nc,
        )
        return tuple(outs)

    devices = jax.devices()[:8]
    mesh = Mesh(np.asarray(devices), ("core",))
    in_specs = (PartitionSpec("core"),) * (n_params + len(out_names))
    out_specs = (PartitionSpec("core"),) * len(out_names)
    jitted = jax.jit(
        shard_map(
            _body, mesh=mesh, in_specs=in_specs, out_specs=out_specs, check_rep=False
        ),
        keep_unused=True,
    )
    zeros = [np.zeros((8 * av.shape[0], *av.shape[1:]), av.dtype) for av in out_avals]
    return (jitted, in_names, out_names, out_avals, zeros, mesh)


def _get_runner(loop_r=None):
    key = ("runner", loop_r)
    if key not in _CACHE:
        _CACHE[key] = _make_runner(loop_r)
    return _CACHE[key]


def _prep_core_inputs(x, wq_w, wq_b, wk_w, wk_b, wv_w, wv_b, dense_w):
    """Per-core host-side shard prep. Returns list of dicts (8 cores)."""
    maps = []
    for c in range(8):
        b, half = c // 2, c % 2
        f0 = half * HHALF
        fs = slice(f0, f0 + HHALF)
        maps.append(
            {
                "xT": np.ascontiguousarray(x[b].T).astype(NPBF16),
                "wqT": np.ascontiguousarray(wq_w[fs].T).astype(NPBF16),
                "wkT": np.ascontiguousarray(wk_w[fs].T).astype(NPBF16),
                "wvT": np.ascontiguousarray(wv_w[fs].T).astype(NPBF16),
                "dwT": np.ascontiguousarray(dense_w[:, fs].T).astype(NPBF16),
                "qb": np.ascontiguousarray(wq_b[fs].reshape(4, P).T.astype(np.float32)),
                "kb": np.ascontiguousarray(wk_b[fs].reshape(4, P).T.astype(np.float32)),
                "vb": np.broadcast_to(
                    wv_b[fs].reshape(1, 8, 64).astype(NPBF16), (P, 8, 64)
                ).copy(),
            }
        )
    return maps


def run_device(in_maps, time_iters=0, loop_r=None):
    """Run the SPMD kernel. Returns (per-core outp list, best wall ns or None)."""
    jitted, in_names, out_names, out_avals, zeros, mesh = _get_runner(loop_r)
    concat_in = [
        np.concatenate([in_maps[c][name] for c in range(8)], axis=0)
        for name in in_names
    ]
    args = concat_in + zeros
    outs = jitted(*args)
    outs = [np.asarray(o) for o in outs]
    best_ns = None
    if time_iters:
        import jax
        from jax.sharding import NamedSharding, PartitionSpec

        sh = NamedSharding(mesh, PartitionSpec("core"))
        dev_args = [jax.device_put(a, sh) for a in args]
        jax.block_until_ready(dev_args)
        times = []
        for _ in range(time_iters):
            t0 = time.perf_counter()
            o = jitted(*dev_args)
            jax.block_until_ready(o)
            times.append(time.perf_counter() - t0)
        best_ns = int(min(times) * 1e9)
    per_core = [
        {
            name: outs[i].reshape(8, *out_avals[i].shape)[c]
            for i, name in enumerate(out_names)
        }
        for c in range(8)
    ]
    return per_core, best_ns


def kernel(**inputs):
    x = np.asarray(inputs["x"], np.float32)
    args = {
        k: np.asarray(inputs[k], np.float32)
        for k in ["wq_w", "wq_b", "wk_w", "wk_b", "wv_w", "wv_b", "dense_w"]
    }
    in_maps = _prep_core_inputs(x, **args)
    per_core, _ = run_device(in_maps)
    dense_b = np.asarray(inputs["dense_b"], np.float32)
    out = np.empty((B, S, D), np.float32)
    for b in range(B):
        out[b] = per_core[2 * b]["outp"] + per_core[2 * b + 1]["outp"] + dense_b
    return out
